# revision 1
# baseline (speedup 1.0000x reference)
"""Trainium2 Bass kernel for Co-occurrence Infused Multi-Label Attention.

Shards the n_classes (code) axis across 8 NeuronCores; [token, class]
orientation so the softmax-weighted token contraction runs on the PE.

Per core (c = class shard of 1152, z = head, b = chunk, t = token):
  QgT [tf, c]  = tanh(trans_wT @ QT + b_tr)     (PE + ACT, tf on partitions)
  qT  [zh, c]  = q_wT @ QgT + q_b               (PE + DVE bias-add)
  QwTplus      = [per-z W_wT @ QgT ; ones]      (PE + DVE copy), [65, z*512+c]
  WKT [zh, t]  = tanh(k_wT @ HT + k_b)          (PE + ACT, bias per partition)
  WVplus [t, z*65+h] = [tanh(HT.T @ v_wT + v_b) ; ones]  (v_b via rank-1 MM)
  per (c-chunk of 512, b, z):
    scoresT[t, c] = WKT_z.T @ qT_z              (4 tchunk MMs, K=64)
    expT          = ACT Exp (PSUM->SBUF bf16, [128, 2w] instrs)
    Y [65, c]     = sum_t WVplus_z.T @ expT     (4 MMs K=128; row 64 = denom)
    prod [65, c]  = Y * QwTplus_z               (DVE, the only big DVE op)
    RD[0:32, 0:w]   += selR_idx.T @ prod        (PE: row idx = numerator)
    RD[0:32, 512:+w] += selD_idx.T @ prod       (PE: row idx = denominator)
  normR = RD[:, 0:w] * recip(RD[:, 512:+w])     (DVE, tiny)
  out [4, c] = zsel.T @ normR                   (PE, sums over z)
"""

import numpy as np
import ml_dtypes

# Problem constants (hardcoded per harness contract)
C_FULL = 8929
D = 768          # d_model
TF = 512         # transform dim (= NH * DK)
NH = 8           # heads
DK = 64          # head dim
B = 4            # chunks
T = 512          # tokens per chunk
BT = B * T       # 2048
N_CORES = 8
CP = 9216        # padded classes (8 * 1152)
CS = CP // N_CORES   # 1152 classes per core
NDC = D // 128       # 6 d-model chunks
NFC = TF // 128      # 4 transform chunks
NTT = BT // 128      # 16 token tiles
NPAIR = B * NH       # 32 (b,z) pairs
C_CHUNKS = [(0, 512), (512, 512), (1024, 128)]  # (offset, width) per core
SELW = 32 * 32 * 2 + 4   # selector tensor width (R blocks, D blocks, zsel)

_BF = ml_dtypes.bfloat16

_CACHE = {}


def _make_sel():
    """Selector constants [65, 2052]: per-pair numerator col blocks
    (cols idx*32..), denominator blocks (cols 1024+idx*32..), and the
    z-sum selector (cols 2048..2052, idx = z*4+b)."""
    sel = np.zeros((65, SELW), np.float32)
    for idx in range(NPAIR):
        sel[0:64, idx * 32 + idx] = 1.0            # numerator: sum rows 0-63
        sel[64, 1024 + idx * 32 + idx] = 1.0       # denominator: row 64
    for r in range(NPAIR):
        sel[r, 2048 + (r % 4)] = 1.0               # z-sum: idx = z*4+b
    return sel.astype(_BF)


def _build(a_zero: bool, reps: int = 1, zpair: bool = False, dma_spread: bool = True):
    from contextlib import ExitStack
    import concourse.bass as bass
    import concourse.mybir as mybir
    import concourse.tile as tile
    from concourse import bacc

    bf = mybir.dt.bfloat16
    f32 = mybir.dt.float32
    AF = mybir.ActivationFunctionType
    ALU = mybir.AluOpType

    nc = bacc.Bacc()

    qt_d = nc.declare_dram_parameter("qt", [D, CS], bf, isOutput=False)
    ht_d = nc.declare_dram_parameter("ht", [D, BT], bf, isOutput=False)
    wtr_d = nc.declare_dram_parameter("wtr", [D, TF], bf, isOutput=False)
    wq_d = nc.declare_dram_parameter("wq", [TF, TF], bf, isOutput=False)
    wk_d = nc.declare_dram_parameter("wk", [D, TF], bf, isOutput=False)
    wv_d = nc.declare_dram_parameter("wv", [D, TF], bf, isOutput=False)
    ww_d = nc.declare_dram_parameter("ww", [TF, TF], bf, isOutput=False)
    btr_d = nc.declare_dram_parameter("btr", [TF], f32, isOutput=False)
    bq_d = nc.declare_dram_parameter("bq", [TF], f32, isOutput=False)
    bk_d = nc.declare_dram_parameter("bk", [TF], f32, isOutput=False)
    bvb_d = nc.declare_dram_parameter("bvb", [1, TF], bf, isOutput=False)
    sel_d = nc.declare_dram_parameter("sel", [65, SELW], bf, isOutput=False)
    ea_d = None
    if not a_zero:
        ea_d = nc.declare_dram_parameter("ea", [128, NTT], f32, isOutput=False)
    out_d = nc.declare_dram_parameter("out", [B, CS], f32, isOutput=True)

    with tile.TileContext(nc) as tc, ExitStack() as top:
        const = top.enter_context(tc.tile_pool(name="const", bufs=1))

        # --- load weights / H / biases / selectors ---
        w_tr = const.tile([128, NDC * TF], bf)
        w_k = const.tile([128, NDC * TF], bf)
        w_v = const.tile([128, NDC * TF], bf)
        _dmae = [nc.sync, nc.scalar] if dma_spread else [nc.sync]
        _dmai = [0]
        def _dma(out, in_):
            _dmae[_dmai[0] % len(_dmae)].dma_start(out, in_)
            _dmai[0] += 1
        for j in range(NDC):
            _dma(w_tr[:, j * TF:(j + 1) * TF], wtr_d[j * 128:(j + 1) * 128, :])
            _dma(w_k[:, j * TF:(j + 1) * TF], wk_d[j * 128:(j + 1) * 128, :])
            _dma(w_v[:, j * TF:(j + 1) * TF], wv_d[j * 128:(j + 1) * 128, :])
        w_q = const.tile([128, NFC * TF], bf)
        w_W = const.tile([128, NFC * TF], bf)
        for j in range(NFC):
            _dma(w_q[:, j * TF:(j + 1) * TF], wq_d[j * 128:(j + 1) * 128, :])
            _dma(w_W[:, j * TF:(j + 1) * TF], ww_d[j * 128:(j + 1) * 128, :])
        ht_sb = const.tile([128, NDC * BT], bf)
        for j in range(NDC):
            _dma(ht_sb[:, j * BT:(j + 1) * BT], ht_d[j * 128:(j + 1) * 128, :])
        b_tr = const.tile([128, NFC], f32)
        b_q = const.tile([128, NFC], f32)
        b_k = const.tile([128, NFC], f32)
        nc.sync.dma_start(b_tr[:], btr_d[:].rearrange("(c p) -> p c", p=128))
        nc.sync.dma_start(b_q[:], bq_d[:].rearrange("(c p) -> p c", p=128))
        nc.sync.dma_start(b_k[:], bk_d[:].rearrange("(c p) -> p c", p=128))
        bvb = const.tile([1, TF], bf)
        nc.sync.dma_start(bvb[:], bvb_d[:, :])
        sel = const.tile([65, SELW], bf)
        nc.sync.dma_start(sel[:], sel_d[:, :])
        ones1 = const.tile([1, 128], bf)
        nc.gpsimd.memset(ones1[:], 1.0)
        ea_sb = None
        if not a_zero:
            ea_sb = const.tile([128, NTT], f32)
            nc.sync.dma_start(ea_sb[:], ea_d[:, :])

        # --- WKT [zh, t] = tanh(k_wT @ HT + k_b) ---
        wkt = const.tile([128, NFC * BT], bf)
        # --- WVplus [t, z*65+h], one [128, 520] block per token tile ---
        wvp = const.tile([128, NTT * 520], bf)
        for jt in range(NTT):
            for z in range(NH):
                nc.gpsimd.memset(wvp[:, jt * 520 + z * 65 + 64: jt * 520 + z * 65 + 65], 1.0)

        with ExitStack() as main:
            qin = main.enter_context(tc.tile_pool(name="qin", bufs=2))
            qg = main.enter_context(tc.tile_pool(name="qg", bufs=2))
            if not zpair:
                chps = main.enter_context(tc.tile_pool(name="chps", bufs=1, space="PSUM"))
            scps = main.enter_context(tc.tile_pool(name="scps", bufs=2, space="PSUM"))
            yps = main.enter_context(tc.tile_pool(name="yps", bufs=2 if zpair else 1, space="PSUM"))
            def chain_ps():
                if zpair:
                    t = scps.tile([128, 1024], f32, tag="psc", name="chainps")
                    return t
                t = chps.tile([128, 512], f32, tag="chain", name="chainps")
                return t
            rdps = main.enter_context(tc.tile_pool(name="rdps", bufs=1, space="PSUM"))
            expp = main.enter_context(tc.tile_pool(name="expp", bufs=6))
            prodp = main.enter_context(tc.tile_pool(name="prodp", bufs=4))
            tailp = main.enter_context(tc.tile_pool(name="tailp", bufs=2))
            outp = main.enter_context(tc.tile_pool(name="outp", bufs=2))

            for rep in range(reps):
                # --- K/V transform (psum slots shared with scores pool) ---
                for jz in range(NFC):
                    for jp in range(BT // 1024):
                        ps = scps.tile([128, 1024], f32, tag="psc")
                        for half in range(2):
                            jt = jp * 2 + half
                            for jd in range(NDC):
                                nc.tensor.matmul(
                                    ps[:, half * 512: half * 512 + 512],
                                    w_k[:, jd * TF + jz * 128: jd * TF + (jz + 1) * 128],
                                    ht_sb[:, jd * BT + jt * 512: jd * BT + (jt + 1) * 512],
                                    start=(jd == 0), stop=(jd == NDC - 1))
                        nc.scalar.activation(
                            wkt[:, jz * BT + jp * 1024: jz * BT + (jp + 1) * 1024],
                            ps[:, 0:1024], AF.Tanh, bias=b_k[:, jz:jz + 1])
                for jt in range(NTT):
                    ps = scps.tile([128, 1024], f32, tag="psc")
                    for jd in range(NDC):
                        nc.tensor.matmul(
                            ps[:, 0:512],
                            ht_sb[:, jd * BT + jt * 128: jd * BT + (jt + 1) * 128],
                            w_v[:, jd * TF:(jd + 1) * TF],
                            start=(jd == 0), stop=False)
                    nc.tensor.matmul(ps[:, 0:512], ones1[0:1, :], bvb[0:1, :],
                                     start=False, stop=True)
                    wvp_z = wvp[:, jt * 520: (jt + 1) * 520].rearrange(
                        "p (z h) -> p z h", h=65)
                    nc.scalar.activation(
                        wvp_z[:, :, 0:64],
                        ps[:, 0:512].rearrange("p (z h) -> p z h", h=64),
                        AF.Tanh)

                for (c0, w) in C_CHUNKS:
                    qt_sb = qin.tile([128, NDC * 512], bf, tag="qt")
                    for jd in range(NDC):
                        nc.sync.dma_start(qt_sb[:, jd * 512: jd * 512 + w],
                                          qt_d[jd * 128:(jd + 1) * 128, c0:c0 + w])
                    # QgT [tf, c] = tanh(trans_wT @ QT + b_tr)
                    qgt = qg.tile([128, NFC * 512], bf, tag="qgt")
                    for jf in range(NFC):
                        ps = chain_ps()
                        for jd in range(NDC):
                            nc.tensor.matmul(
                                ps[:, :w],
                                w_tr[:, jd * TF + jf * 128: jd * TF + (jf + 1) * 128],
                                qt_sb[:, jd * 512: jd * 512 + w],
                                start=(jd == 0), stop=(jd == NDC - 1))
                        nc.scalar.activation(qgt[:, jf * 512: jf * 512 + w], ps[:, :w],
                                             AF.Tanh, bias=b_tr[:, jf:jf + 1])
                    # qT [zh, c] = q_wT @ QgT + q_b  (bias-add on DVE)
                    qtt = qg.tile([128, NFC * 512], bf, tag="qtt")
                    for jz in range(NFC):
                        ps = chain_ps()
                        for jf in range(NFC):
                            nc.tensor.matmul(
                                ps[:, :w],
                                w_q[:, jf * TF + jz * 128: jf * TF + (jz + 1) * 128],
                                qgt[:, jf * 512: jf * 512 + w],
                                start=(jf == 0), stop=(jf == NFC - 1))
                        nc.vector.tensor_scalar_add(qtt[:, jz * 512: jz * 512 + w],
                                                    ps[:, :w], b_q[:, jz:jz + 1])
                    # QwTplus [65, z*512+c]: rows 0-63 per-z W_wT@QgT, row 64 ones
                    qwtp = qg.tile([65, NH * 512], bf, tag="qwtp")
                    nc.gpsimd.memset(qwtp[64:65, :], 1.0)
                    for z in range(NH):
                        jz, hz = z // 2, (z % 2) * 64
                        ps = chain_ps()
                        for jf in range(NFC):
                            nc.tensor.matmul(
                                ps[0:64, :w],
                                w_W[:, jf * TF + jz * 128 + hz: jf * TF + jz * 128 + hz + 64],
                                qgt[:, jf * 512: jf * 512 + w],
                                start=(jf == 0), stop=(jf == NFC - 1))
                        nc.vector.tensor_copy(qwtp[0:64, z * 512: z * 512 + w],
                                              ps[0:64, :w])

                    # attention pairs
                    rd = rdps.tile([32, 1024], f32, tag="rd")
                    if zpair:
                        for step in range(NPAIR // 2):
                            bb = step // (NH // 2)
                            jz = step % (NH // 2)
                            ys = [None, None]
                            for half in range(2):
                                psc_a = scps.tile([128, 1024], f32, tag="psc")
                                psc_b = scps.tile([128, 1024], f32, tag="psc")
                                pscs = [psc_a, psc_b]
                                for slot in range(2):
                                    jt = half * 2 + slot
                                    for zi in range(2):
                                        hz = zi * 64
                                        nc.tensor.matmul(
                                            pscs[zi][:, slot * w: slot * w + w],
                                            wkt[hz:hz + 64,
                                                jz * BT + bb * 512 + jt * 128:
                                                jz * BT + bb * 512 + (jt + 1) * 128],
                                            qtt[hz:hz + 64, jz * 512: jz * 512 + w],
                                            start=True, stop=True)
                                for zi in range(2):
                                    z = jz * 2 + zi
                                    et = expp.tile([128, 1024], bf, tag="et")
                                    nc.scalar.activation(et[:, 0:2 * w],
                                                         pscs[zi][:, 0:2 * w], AF.Exp)
                                    if not a_zero:
                                        et2 = expp.tile([128, 1024], bf, tag="et2")
                                        for slot in range(2):
                                            jt = half * 2 + slot
                                            nc.vector.tensor_scalar_mul(
                                                et2[:, slot * w: slot * w + w],
                                                et[:, slot * w: slot * w + w],
                                                ea_sb[:, bb * 4 + jt: bb * 4 + jt + 1])
                                        et = et2
                                    if half == 0:
                                        yv = yps.tile([65, 512], f32, tag="y")
                                        ys[zi] = yv
                                    for slot in range(2):
                                        jt = half * 2 + slot
                                        gt = bb * 4 + jt
                                        nc.tensor.matmul(
                                            ys[zi][:, :w],
                                            wvp[:, gt * 520 + z * 65: gt * 520 + (z + 1) * 65],
                                            et[:, slot * w: slot * w + w],
                                            start=(jt == 0), stop=(jt == 3))
                            for zi in range(2):
                                z = jz * 2 + zi
                                idx = z * B + bb
                                prod = prodp.tile([65, 512], bf, tag="prod")
                                nc.vector.tensor_mul(prod[:, :w], ys[zi][:, :w],
                                                     qwtp[:, z * 512: z * 512 + w])
                                first = (step == 0 and zi == 0)
                                last = (step == NPAIR // 2 - 1 and zi == 1)
                                nc.tensor.matmul(rd[:, 0:w],
                                                 sel[:, idx * 32: (idx + 1) * 32],
                                                 prod[:, :w],
                                                 start=first, stop=last)
                                nc.tensor.matmul(rd[:, 512: 512 + w],
                                                 sel[:, 1024 + idx * 32: 1024 + (idx + 1) * 32],
                                                 prod[:, :w],
                                                 start=first, stop=last)
                    else:
                     for pair in range(NPAIR):
                        z = pair % NH
                        bb = pair // NH
                        jz, hz = z // 2, (z % 2) * 64
                        idx = z * B + bb
                        for half in range(2):
                            psc = scps.tile([128, 1024], f32, tag="psc")
                            for slot in range(2):
                                jt = half * 2 + slot
                                nc.tensor.matmul(
                                    psc[:, slot * w: slot * w + w],
                                    wkt[hz:hz + 64,
                                        jz * BT + bb * 512 + jt * 128:
                                        jz * BT + bb * 512 + (jt + 1) * 128],
                                    qtt[hz:hz + 64, jz * 512: jz * 512 + w],
                                    start=True, stop=True)
                            et = expp.tile([128, 1024], bf, tag="et")
                            nc.scalar.activation(et[:, 0:2 * w], psc[:, 0:2 * w], AF.Exp)
                            if not a_zero:
                                et2 = expp.tile([128, 1024], bf, tag="et2")
                                for slot in range(2):
                                    jt = half * 2 + slot
                                    nc.vector.tensor_scalar_mul(
                                        et2[:, slot * w: slot * w + w],
                                        et[:, slot * w: slot * w + w],
                                        ea_sb[:, bb * 4 + jt: bb * 4 + jt + 1])
                                et = et2
                            if half == 0:
                                y = yps.tile([65, 512], f32, tag="y")
                            for slot in range(2):
                                jt = half * 2 + slot
                                gt = bb * 4 + jt
                                nc.tensor.matmul(
                                    y[:, :w],
                                    wvp[:, gt * 520 + z * 65: gt * 520 + (z + 1) * 65],
                                    et[:, slot * w: slot * w + w],
                                    start=(jt == 0), stop=(jt == 3))
                        prod = prodp.tile([65, 512], bf, tag="prod")
                        nc.vector.tensor_mul(prod[:, :w], y[:, :w],
                                             qwtp[:, z * 512: z * 512 + w])
                        nc.tensor.matmul(rd[:, 0:w],
                                         sel[:, idx * 32: (idx + 1) * 32],
                                         prod[:, :w],
                                         start=(pair == 0), stop=(pair == NPAIR - 1))
                        nc.tensor.matmul(rd[:, 512: 512 + w],
                                         sel[:, 1024 + idx * 32: 1024 + (idx + 1) * 32],
                                         prod[:, :w],
                                         start=(pair == 0), stop=(pair == NPAIR - 1))

                    # tail: normalize and z-sum
                    rden = tailp.tile([32, 512], f32, tag="rden")
                    nc.vector.reciprocal(rden[:, :w], rd[:, 512: 512 + w])
                    normr = tailp.tile([32, 512], bf, tag="normr")
                    nc.vector.tensor_mul(normr[:, :w], rd[:, 0:w], rden[:, :w])
                    if zpair:
                        zs = rdps.tile([32, 1024], f32, tag="rd")
                    else:
                        zs = chps.tile([128, 512], f32, tag="chain")
                    nc.tensor.matmul(zs[0:4, :w], sel[0:32, 2048:2052], normr[:, :w],
                                     start=True, stop=True)
                    ot = outp.tile([4, 512], f32, tag="ot")
                    nc.vector.tensor_copy(ot[:, :w], zs[0:4, :w])
                    nc.sync.dma_start(out_d[:, c0:c0 + w], ot[:, :w])

    nc.compile()
    return nc


def _get_nc(a_zero: bool):
    key = ("nc", a_zero)
    if key not in _CACHE:
        _CACHE[key] = _build(a_zero)
    return _CACHE[key]


def _prep_inputs(Q, H, a, trans_w, trans_b, q_w, q_b, k_w, k_b, v_w, v_b, W_w):
    """Host-side sharding/layout. Returns (in_maps, a_zero)."""
    a = np.asarray(a, np.float32)
    a_zero = not np.any(a)

    qt_full = np.zeros((D, CP), _BF)
    qt_full[:, :C_FULL] = np.asarray(Q, np.float32).T.astype(_BF)
    ht = np.ascontiguousarray(
        np.asarray(H, np.float32).reshape(BT, D).T.astype(_BF))
    shared = {
        "ht": ht,
        "wtr": np.ascontiguousarray(np.asarray(trans_w, np.float32).T.astype(_BF)),
        "wq": np.ascontiguousarray(np.asarray(q_w, np.float32).T.astype(_BF)),
        "wk": np.ascontiguousarray(np.asarray(k_w, np.float32).T.astype(_BF)),
        "wv": np.ascontiguousarray(np.asarray(v_w, np.float32).T.astype(_BF)),
        "ww": np.ascontiguousarray(np.asarray(W_w, np.float32).T.astype(_BF)),
        "btr": np.asarray(trans_b, np.float32),
        "bq": np.asarray(q_b, np.float32),
        "bk": np.asarray(k_b, np.float32),
        "bvb": np.asarray(v_b, np.float32).reshape(1, TF).astype(_BF),
        "sel": _make_sel(),
    }
    if not a_zero:
        ea = np.exp(a).reshape(B, 4, 128).transpose(2, 0, 1).reshape(128, NTT)
        shared["ea"] = np.ascontiguousarray(ea.astype(np.float32))
    in_maps = []
    for c in range(N_CORES):
        m = dict(shared)
        m["qt"] = np.ascontiguousarray(qt_full[:, c * CS:(c + 1) * CS])
        in_maps.append(m)
    return in_maps, a_zero


def kernel(**inputs) -> np.ndarray:
    from concourse.bass_utils import run_bass_kernel_spmd

    in_maps, a_zero = _prep_inputs(**inputs)
    nc = _get_nc(a_zero)
    res = run_bass_kernel_spmd(nc, in_maps, list(range(N_CORES)))
    out = np.concatenate([res.results[c]["out"] for c in range(N_CORES)], axis=1)
    return np.ascontiguousarray(out[:, :C_FULL])



# revision 2
# speedup vs baseline: 13.4604x; 13.4604x over previous
"""Trainium2 Bass kernel for Co-occurrence Infused Multi-Label Attention.

Shards the n_classes (code) axis across 8 NeuronCores; [token, class]
orientation so the softmax-weighted token contraction runs on the PE.

Per core (c = class shard of 1120, z = head, b = chunk, t = token):
  QgT [tf, c]  = tanh(trans_wT @ QT + b_tr)     (PE + ACT, tf on partitions)
  qT  [zh, c]  = q_wT @ QgT + q_b               (PE + DVE bias-add)
  QwTplus      = [per-z W_wT @ QgT ; ones]      (PE M=128 z-pairs + DVE copy)
  WKT [zh, t]  = tanh(k_wT @ HT + k_b)          (PE + ACT, bias per partition)
  WVplus [t, z*65+h] = [tanh(HT.T @ v_wT + v_b) ; ones]  (v_b via rank-1 MM)
  per (c-chunk, b, z):
    scoresT[t, c] = WKT_z.T @ qT_z              (4 tchunk MMs, K=64)
    expT          = ACT Exp (PSUM->SBUF bf16, [128, 2w] instrs)
    Y [65, c]     = sum_t WVplus_z.T @ expT     (4 MMs K=128; row 64 = denom)
    prod [65, c]  = Y * QwTplus_z               (DVE, the only big DVE op)
    RD[0:64, :w]  += sel_idx.T @ prod           (PE: one MM; rows 0-31 numer,
                                                 rows 32-63 denom)
  normR = RD[0:32] * recip(RD[32:64])           (DVE, tiny)
  out [4, c] = zsel.T @ normR                   (PE, sums over z)
"""

import numpy as np
import ml_dtypes

# Problem constants (hardcoded per harness contract)
C_FULL = 8929
D = 768          # d_model
TF = 512         # transform dim (= NH * DK)
NH = 8           # heads
DK = 64          # head dim
B = 4            # chunks
T = 512          # tokens per chunk
BT = B * T       # 2048
N_CORES = 8
CP = 8960        # padded classes (8 * 1120)
CS = CP // N_CORES   # 1120 classes per core
NDC = D // 128       # 6 d-model chunks
NFC = TF // 128      # 4 transform chunks
NTT = BT // 128      # 16 token tiles
NPAIR = B * NH       # 32 (b,z) pairs
C_CHUNKS = [(0, 512), (512, 512), (1024, 96)]  # (offset, width) per core
SELW = NPAIR * 64 + 4   # selector width (merged R|D blocks + zsel)

_BF = ml_dtypes.bfloat16

_CACHE = {}


def _make_sel():
    """Selector constants [65, 2052]: per-pair merged numer/denom block
    (cols idx*64..idx*64+64: numer -> row idx via col idx, denom -> row
    32+idx via col 32+idx), and the z-sum selector (cols 2048..2052,
    idx = z*4+b)."""
    sel = np.zeros((65, SELW), np.float32)
    for idx in range(NPAIR):
        sel[0:64, idx * 64 + idx] = 1.0            # numerator: sum rows 0-63
        sel[64, idx * 64 + 32 + idx] = 1.0         # denominator: row 64
    for r in range(NPAIR):
        sel[r, NPAIR * 64 + (r % 4)] = 1.0         # z-sum: idx = z*4+b
    return sel.astype(_BF)


def _build(a_zero: bool, reps: int = 1):
    from contextlib import ExitStack
    import concourse.bass as bass
    import concourse.mybir as mybir
    import concourse.tile as tile
    from concourse import bacc

    bf = mybir.dt.bfloat16
    f32 = mybir.dt.float32
    AF = mybir.ActivationFunctionType
    ALU = mybir.AluOpType

    nc = bacc.Bacc()

    qt_d = nc.declare_dram_parameter("qt", [D, CS], bf, isOutput=False)
    ht_d = nc.declare_dram_parameter("ht", [D, BT], bf, isOutput=False)
    wtr_d = nc.declare_dram_parameter("wtr", [D, TF], bf, isOutput=False)
    wq_d = nc.declare_dram_parameter("wq", [TF, TF], bf, isOutput=False)
    wk_d = nc.declare_dram_parameter("wk", [D, TF], bf, isOutput=False)
    wv_d = nc.declare_dram_parameter("wv", [D, TF], bf, isOutput=False)
    ww_d = nc.declare_dram_parameter("ww", [TF, TF], bf, isOutput=False)
    btr_d = nc.declare_dram_parameter("btr", [TF], f32, isOutput=False)
    bq_d = nc.declare_dram_parameter("bq", [TF], f32, isOutput=False)
    bk_d = nc.declare_dram_parameter("bk", [TF], f32, isOutput=False)
    bvb_d = nc.declare_dram_parameter("bvb", [1, TF], bf, isOutput=False)
    sel_d = nc.declare_dram_parameter("sel", [65, SELW], bf, isOutput=False)
    ea_d = None
    if not a_zero:
        ea_d = nc.declare_dram_parameter("ea", [128, NTT], f32, isOutput=False)
    out_d = nc.declare_dram_parameter("out", [B, CS], f32, isOutput=True)

    with tile.TileContext(nc) as tc, ExitStack() as top:
        const = top.enter_context(tc.tile_pool(name="const", bufs=1))

        # --- load weights / H / biases / selectors ---
        w_tr = const.tile([128, NDC * TF], bf)
        w_k = const.tile([128, NDC * TF], bf)
        w_v = const.tile([128, NDC * TF], bf)
        _dmae = [nc.sync, nc.scalar]
        _dmai = [0]
        def _dma(out, in_):
            _dmae[_dmai[0] % len(_dmae)].dma_start(out, in_)
            _dmai[0] += 1
        for j in range(NDC):
            _dma(w_tr[:, j * TF:(j + 1) * TF], wtr_d[j * 128:(j + 1) * 128, :])
            _dma(w_k[:, j * TF:(j + 1) * TF], wk_d[j * 128:(j + 1) * 128, :])
            _dma(w_v[:, j * TF:(j + 1) * TF], wv_d[j * 128:(j + 1) * 128, :])
        w_q = const.tile([128, NFC * TF], bf)
        w_W = const.tile([128, NFC * TF], bf)
        for j in range(NFC):
            _dma(w_q[:, j * TF:(j + 1) * TF], wq_d[j * 128:(j + 1) * 128, :])
            _dma(w_W[:, j * TF:(j + 1) * TF], ww_d[j * 128:(j + 1) * 128, :])
        ht_sb = const.tile([128, NDC * BT], bf)
        for j in range(NDC):
            _dma(ht_sb[:, j * BT:(j + 1) * BT], ht_d[j * 128:(j + 1) * 128, :])
        b_tr = const.tile([128, NFC], f32)
        b_q = const.tile([128, NFC], f32)
        b_k = const.tile([128, NFC], f32)
        nc.sync.dma_start(b_tr[:], btr_d[:].rearrange("(c p) -> p c", p=128))
        nc.sync.dma_start(b_q[:], bq_d[:].rearrange("(c p) -> p c", p=128))
        nc.sync.dma_start(b_k[:], bk_d[:].rearrange("(c p) -> p c", p=128))
        bvb = const.tile([1, TF], bf)
        nc.sync.dma_start(bvb[:], bvb_d[:, :])
        sel = const.tile([65, SELW], bf)
        nc.sync.dma_start(sel[:], sel_d[:, :])
        ones1 = const.tile([1, 128], bf)
        nc.gpsimd.memset(ones1[:], 1.0)
        ea_sb = None
        if not a_zero:
            ea_sb = const.tile([128, NTT], f32)
            nc.sync.dma_start(ea_sb[:], ea_d[:, :])

        # --- WKT [zh, t] = tanh(k_wT @ HT + k_b) ---
        wkt = const.tile([128, NFC * BT], bf)
        # --- WVplus [t, z*65+h], one [128, 520] block per token tile ---
        wvp = const.tile([128, NTT * 520], bf)
        for jt in range(NTT):
            for z in range(NH):
                nc.gpsimd.memset(wvp[:, jt * 520 + z * 65 + 64: jt * 520 + z * 65 + 65], 1.0)

        with ExitStack() as main:
            qin = main.enter_context(tc.tile_pool(name="qin", bufs=2))
            qg = main.enter_context(tc.tile_pool(name="qg", bufs=2))
            chps = main.enter_context(tc.tile_pool(name="chps", bufs=1, space="PSUM"))
            scps = main.enter_context(tc.tile_pool(name="scps", bufs=2, space="PSUM"))
            yps = main.enter_context(tc.tile_pool(name="yps", bufs=2, space="PSUM"))
            rdps = main.enter_context(tc.tile_pool(name="rdps", bufs=1, space="PSUM"))
            expp = main.enter_context(tc.tile_pool(name="expp", bufs=6))
            prodp = main.enter_context(tc.tile_pool(name="prodp", bufs=4))
            tailp = main.enter_context(tc.tile_pool(name="tailp", bufs=2))
            outp = main.enter_context(tc.tile_pool(name="outp", bufs=2))

            def chain_ps():
                return chps.tile([128, 512], f32, tag="chain", name="chainps")

            for rep in range(reps):
                # --- K/V transform (psum slots shared with scores pool) ---
                for jz in range(NFC):
                    for jp in range(BT // 1024):
                        ps = scps.tile([128, 1024], f32, tag="psc")
                        for half in range(2):
                            jt = jp * 2 + half
                            for jd in range(NDC):
                                nc.tensor.matmul(
                                    ps[:, half * 512: half * 512 + 512],
                                    w_k[:, jd * TF + jz * 128: jd * TF + (jz + 1) * 128],
                                    ht_sb[:, jd * BT + jt * 512: jd * BT + (jt + 1) * 512],
                                    start=(jd == 0), stop=(jd == NDC - 1))
                        nc.scalar.activation(
                            wkt[:, jz * BT + jp * 1024: jz * BT + (jp + 1) * 1024],
                            ps[:, 0:1024], AF.Tanh, bias=b_k[:, jz:jz + 1])
                for jt in range(NTT):
                    ps = scps.tile([128, 1024], f32, tag="psc")
                    for jd in range(NDC):
                        nc.tensor.matmul(
                            ps[:, 0:512],
                            ht_sb[:, jd * BT + jt * 128: jd * BT + (jt + 1) * 128],
                            w_v[:, jd * TF:(jd + 1) * TF],
                            start=(jd == 0), stop=False)
                    nc.tensor.matmul(ps[:, 0:512], ones1[0:1, :], bvb[0:1, :],
                                     start=False, stop=True)
                    wvp_z = wvp[:, jt * 520: (jt + 1) * 520].rearrange(
                        "p (z h) -> p z h", h=65)
                    nc.scalar.activation(
                        wvp_z[:, :, 0:64],
                        ps[:, 0:512].rearrange("p (z h) -> p z h", h=64),
                        AF.Tanh)

                for (c0, w) in C_CHUNKS:
                    qt_sb = qin.tile([128, NDC * 512], bf, tag="qt")
                    for jd in range(NDC):
                        nc.sync.dma_start(qt_sb[:, jd * 512: jd * 512 + w],
                                          qt_d[jd * 128:(jd + 1) * 128, c0:c0 + w])
                    # QgT [tf, c] = tanh(trans_wT @ QT + b_tr)
                    qgt = qg.tile([128, NFC * 512], bf, tag="qgt")
                    for jf in range(NFC):
                        ps = chain_ps()
                        for jd in range(NDC):
                            nc.tensor.matmul(
                                ps[:, :w],
                                w_tr[:, jd * TF + jf * 128: jd * TF + (jf + 1) * 128],
                                qt_sb[:, jd * 512: jd * 512 + w],
                                start=(jd == 0), stop=(jd == NDC - 1))
                        nc.scalar.activation(qgt[:, jf * 512: jf * 512 + w], ps[:, :w],
                                             AF.Tanh, bias=b_tr[:, jf:jf + 1])
                    # qT [zh, c] = q_wT @ QgT + q_b  (bias-add on DVE)
                    qtt = qg.tile([128, NFC * 512], bf, tag="qtt")
                    for jz in range(NFC):
                        ps = chain_ps()
                        for jf in range(NFC):
                            nc.tensor.matmul(
                                ps[:, :w],
                                w_q[:, jf * TF + jz * 128: jf * TF + (jz + 1) * 128],
                                qgt[:, jf * 512: jf * 512 + w],
                                start=(jf == 0), stop=(jf == NFC - 1))
                        nc.vector.tensor_scalar_add(qtt[:, jz * 512: jz * 512 + w],
                                                    ps[:, :w], b_q[:, jz:jz + 1])
                    # QwTplus [65, z*512+c]: rows 0-63 per-z W_wT@QgT, row 64 ones.
                    # One M=128 matmul per z-pair, split into qwtp via DVE.
                    qwtp = qg.tile([65, NH * 512], bf, tag="qwtp")
                    nc.gpsimd.memset(qwtp[64:65, :], 1.0)
                    for jz in range(NFC):
                        ps = chain_ps()
                        for jf in range(NFC):
                            nc.tensor.matmul(
                                ps[:, :w],
                                w_W[:, jf * TF + jz * 128: jf * TF + (jz + 1) * 128],
                                qgt[:, jf * 512: jf * 512 + w],
                                start=(jf == 0), stop=(jf == NFC - 1))
                        nc.vector.tensor_copy(
                            qwtp[0:64, (2 * jz) * 512: (2 * jz) * 512 + w],
                            ps[0:64, :w])
                        nc.vector.tensor_copy(
                            qwtp[0:64, (2 * jz + 1) * 512: (2 * jz + 1) * 512 + w],
                            ps[64:128, :w])

                    # attention pairs
                    rd = rdps.tile([64, 512], f32, tag="rd")
                    for pair in range(NPAIR):
                        z = pair % NH
                        bb = pair // NH
                        jz, hz = z // 2, (z % 2) * 64
                        idx = z * B + bb
                        for half in range(2):
                            psc = scps.tile([128, 1024], f32, tag="psc")
                            for slot in range(2):
                                jt = half * 2 + slot
                                nc.tensor.matmul(
                                    psc[:, slot * w: slot * w + w],
                                    wkt[hz:hz + 64,
                                        jz * BT + bb * 512 + jt * 128:
                                        jz * BT + bb * 512 + (jt + 1) * 128],
                                    qtt[hz:hz + 64, jz * 512: jz * 512 + w],
                                    start=True, stop=True)
                            et = expp.tile([128, 1024], bf, tag="et")
                            nc.scalar.activation(et[:, 0:2 * w], psc[:, 0:2 * w], AF.Exp)
                            if not a_zero:
                                et2 = expp.tile([128, 1024], bf, tag="et2")
                                for slot in range(2):
                                    jt = half * 2 + slot
                                    nc.vector.tensor_scalar_mul(
                                        et2[:, slot * w: slot * w + w],
                                        et[:, slot * w: slot * w + w],
                                        ea_sb[:, bb * 4 + jt: bb * 4 + jt + 1])
                                et = et2
                            if half == 0:
                                y = yps.tile([65, 512], f32, tag="y")
                            for slot in range(2):
                                jt = half * 2 + slot
                                gt = bb * 4 + jt
                                nc.tensor.matmul(
                                    y[:, :w],
                                    wvp[:, gt * 520 + z * 65: gt * 520 + (z + 1) * 65],
                                    et[:, slot * w: slot * w + w],
                                    start=(jt == 0), stop=(jt == 3))
                        prod = prodp.tile([65, 512], bf, tag="prod")
                        nc.vector.tensor_mul(prod[:, :w], y[:, :w],
                                             qwtp[:, z * 512: z * 512 + w])
                        nc.tensor.matmul(rd[0:64, 0:w],
                                         sel[:, idx * 64: (idx + 1) * 64],
                                         prod[:, :w],
                                         start=(pair == 0), stop=(pair == NPAIR - 1))

                    # tail: normalize and z-sum
                    rden = tailp.tile([32, 512], f32, tag="rden")
                    nc.vector.reciprocal(rden[:, :w], rd[32:64, 0:w])
                    normr = tailp.tile([32, 512], bf, tag="normr")
                    nc.vector.tensor_mul(normr[:, :w], rd[0:32, 0:w], rden[:, :w])
                    zs = chps.tile([128, 512], f32, tag="chain", name="chainps")
                    nc.tensor.matmul(zs[0:4, :w], sel[0:32, NPAIR * 64: NPAIR * 64 + 4],
                                     normr[:, :w], start=True, stop=True)
                    ot = outp.tile([4, 512], f32, tag="ot")
                    nc.vector.tensor_copy(ot[:, :w], zs[0:4, :w])
                    nc.sync.dma_start(out_d[:, c0:c0 + w], ot[:, :w])

    nc.compile()
    return nc


def _get_nc(a_zero: bool):
    key = ("nc", a_zero)
    if key not in _CACHE:
        _CACHE[key] = _build(a_zero)
    return _CACHE[key]


def _prep_inputs(Q, H, a, trans_w, trans_b, q_w, q_b, k_w, k_b, v_w, v_b, W_w):
    """Host-side sharding/layout. Returns (in_maps, a_zero)."""
    a = np.asarray(a, np.float32)
    a_zero = not np.any(a)

    qt_full = np.zeros((D, CP), _BF)
    qt_full[:, :C_FULL] = np.asarray(Q, np.float32).T.astype(_BF)
    ht = np.ascontiguousarray(
        np.asarray(H, np.float32).reshape(BT, D).T.astype(_BF))
    shared = {
        "ht": ht,
        "wtr": np.ascontiguousarray(np.asarray(trans_w, np.float32).T.astype(_BF)),
        "wq": np.ascontiguousarray(np.asarray(q_w, np.float32).T.astype(_BF)),
        "wk": np.ascontiguousarray(np.asarray(k_w, np.float32).T.astype(_BF)),
        "wv": np.ascontiguousarray(np.asarray(v_w, np.float32).T.astype(_BF)),
        "ww": np.ascontiguousarray(np.asarray(W_w, np.float32).T.astype(_BF)),
        "btr": np.asarray(trans_b, np.float32),
        "bq": np.asarray(q_b, np.float32),
        "bk": np.asarray(k_b, np.float32),
        "bvb": np.asarray(v_b, np.float32).reshape(1, TF).astype(_BF),
        "sel": _make_sel(),
    }
    if not a_zero:
        ea = np.exp(a).reshape(B, 4, 128).transpose(2, 0, 1).reshape(128, NTT)
        shared["ea"] = np.ascontiguousarray(ea.astype(np.float32))
    in_maps = []
    for c in range(N_CORES):
        m = dict(shared)
        m["qt"] = np.ascontiguousarray(qt_full[:, c * CS:(c + 1) * CS])
        in_maps.append(m)
    return in_maps, a_zero


def kernel(**inputs) -> np.ndarray:
    from concourse.bass_utils import run_bass_kernel_spmd

    in_maps, a_zero = _prep_inputs(**inputs)
    nc = _get_nc(a_zero)
    res = run_bass_kernel_spmd(nc, in_maps, list(range(N_CORES)))
    out = np.concatenate([res.results[c]["out"] for c in range(N_CORES)], axis=1)
    return np.ascontiguousarray(out[:, :C_FULL])
